# revision 38
# baseline (speedup 1.0000x reference)
"""Supervised contrastive loss on 8 Trainium2 NeuronCores — class-sorted
symmetric scheme with per-core column rotation.

Reference (N=8192, D=128, TAU=0.1, 100 classes):
    xn   = x / ||x||_row
    sim  = xn @ xn.T                      [N, N]
    e    = exp(sim / TAU)
    top  = sum_j e[i,j] * (y_i == y_j)
    down = sum_j e[i,j]
    loss = mean(log(down) - log(top))

The loss is a mean over rows, so it is invariant under a joint
row/column permutation. The host sorts samples by class; same-class
pairs then live in a narrow block-diagonal band (class size <= 128).
Additionally each core's copy of the column data (xnT / y_bcast) has
every 1024-column half rotated left by w0 = max(0, 128*(core-1)), which
lands that core's diagonal band at tile columns [0, 384) for quadrant
jobs and [256, 384) for full jobs — so the masked "top" reductions are
384/128 wide instead of 1024/2048.

  - e is symmetric: only ~56% of N^2 computed (upper 128-row-block
    triangle; each 2048^2 diagonal square split into A (top-left
    1024^2), D (bottom-right), B (top-right) handled per-core).
  - down rows: Act accum on full-h0 halves; GPSIMD tensor_scalar accum
    on full-h1 halves and B (GpSimd is otherwise idle). A/D rows get
    down from column sums by square symmetry.
  - down cols: M=128 matmul per 512-chunk (lhsT cols: [ones, k0-ind,
    k8-ind, 0...]) into a 4-bank PSUM group per G; one [4, 2048] DVE
    copy + DMA per G.
  - top cols (mirror): covered by symmetry inside the diagonal squares
    except at the two 1024-aligned boundaries of each square, where a
    straddling class's mirror block is credited via the indicator rows
    (partitions 1 / 2) of the colsum output.

Host does the final log/mean, rotation unscrambling, and all gathers.
"""

import sys

import numpy as np

sys.path.insert(0, "/opt/trn_rl_repo")

import ml_dtypes

TAU = 0.1
N, D = 8192, 128
P = 128
NCORES = 8
CH = 2048                   # column group width
HW2 = CH // 2               # quadrant width
NG = 4                      # column groups
NJOBS = 24                  # jobs per core: 12 full + 4*(B,A,D)
MM_N = 512                  # one PSUM bank of f32 per matmul
W_STT = 384                 # quadrant STT window (rotated band)
POKE0, POKE1 = 256, 384     # full-job STT window (rotated square boundary)

_PROGRAM = None


def rot0(core):
    return max(0, 128 * (core - 1))


def job_table(core):
    """Uniform job list: (rho, G, kind, half); same structure on every core."""
    jobs = []
    for G in (3, 2, 1, 0):
        for r in [r for r in range(16 * G) if r % 8 == core]:
            jobs.append((r, G, "full", None))
        jobs.append((16 * G + core, G, "B", 1))
        jobs.append((16 * G + core, G, "A", 0))
        jobs.append((16 * G + 8 + core, G, "D", 1))
    return jobs


_JOBS0 = job_table(0)
assert len(_JOBS0) == NJOBS


def _build_program():
    import concourse.bacc as bacc
    import concourse.bass as bass  # noqa: F401
    import concourse.mybir as mybir
    from concourse.tile import TileContext

    f32 = mybir.dt.float32
    bf16 = mybir.dt.bfloat16
    AF = mybir.ActivationFunctionType
    OP = mybir.AluOpType

    nc = bacc.Bacc("TRN2", target_bir_lowering=False)
    xnT_h = nc.declare_dram_parameter("xnT", [P, N], bf16, isOutput=False)
    xiT_h = nc.declare_dram_parameter("xiT", [P, NJOBS * P], bf16, isOutput=False)
    # per-job patch indicator cols (cols 1,2 of each job's lhsT): tiny
    ypt_h = nc.declare_dram_parameter("ypatch", [P, NJOBS * 2], bf16, isOutput=False)
    # y_bcast shipped as the 384-wide STT windows only (per G, per half)
    yb_h = nc.declare_dram_parameter("y_bcast", [P, NG * 2 * W_STT], bf16, isOutput=False)
    yi_h = nc.declare_dram_parameter("y_own", [P, NJOBS], f32, isOutput=False)
    out_h = nc.declare_dram_parameter("out", [P, 3 * NJOBS], f32, isOutput=True)
    cls_h = nc.declare_dram_parameter("cls_out", [4, NG * CH], f32, isOutput=True)

    with TileContext(nc) as tc:
        with tc.tile_pool(name="persist", bufs=1) as persist:
            # xnT per G as 4 chunk tiles of 512 so the first matmul can
            # start as soon as the first 128KB lands.
            xnT = {
                g: persist.tile([P, CH], bf16, name=f"xnT{g}")
                for g in range(NG)
            }
            # ybc per G as two 384-wide STT windows (A-half, BD-half)
            ybc = {
                g: persist.tile([P, 2 * W_STT], bf16, name=f"ybc{g}")
                for g in range(NG)
            }
            xiT = persist.tile([P, NJOBS * P], bf16)
            ycl = persist.tile([P, NJOBS * P], bf16)
            ypt = persist.tile([P, NJOBS * 2], bf16)
            yis = persist.tile([P, NJOBS], f32)
            outs = persist.tile([P, 3 * NJOBS], f32)
            warm = persist.tile([P, 8], f32)

            # Warm the exp activation table during the DMA prologue.
            nc.vector.memset(warm[:, 0:4], 0.0)
            nc.scalar.activation(
                out=warm[:, 4:8], in_=warm[:, 0:4], func=AF.Exp, scale=1.0 / TAU
            )

            HD = 2 * P  # first two jobs' stationary rows, loaded first
            nc.sync.dma_start(
                out=xnT[3][:, :HW2], in_=xnT_h[:, 3 * CH : 3 * CH + HW2]
            )
            nc.sync.dma_start(out=xiT[:, :HD], in_=xiT_h[:, :HD])
            nc.sync.dma_start(out=yis[:], in_=yi_h[:, :])
            nc.sync.dma_start(out=ypt[:], in_=ypt_h[:, :])
            nc.sync.dma_start(
                out=xnT[3][:, HW2:], in_=xnT_h[:, 3 * CH + HW2 : 4 * CH]
            )

            # Build the colsum lhsT on-chip: zeros + a strided ones col
            # per job + the two patch-indicator cols (DVE scatter from the
            # contiguous ypt tile — a strided DMA here costs thousands of
            # descriptors and hogs the queue).
            ycl3 = ycl[:].rearrange("p (j c) -> p j c", j=NJOBS)
            nc.vector.memset(ycl[:], 0.0)
            nc.vector.memset(ycl3[:, :, 0:1], 1.0)
            nc.vector.tensor_copy(
                out=ycl3[:, :, 1:3],
                in_=ypt[:].rearrange("p (j c) -> p j c", j=NJOBS),
            )
            first = True
            for g in (3, 2, 1, 0):
                if g != 3:
                    nc.sync.dma_start(
                        out=xnT[g][:], in_=xnT_h[:, g * CH : (g + 1) * CH]
                    )
                nc.sync.dma_start(
                    out=ybc[g][:],
                    in_=yb_h[:, 2 * g * W_STT : 2 * (g + 1) * W_STT],
                )
                if first:
                    nc.sync.dma_start(out=xiT[:, HD:], in_=xiT_h[:, HD:])
                    first = False

            with (
                tc.tile_pool(name="mpsum", bufs=2, space="PSUM") as mpp,
                tc.tile_pool(name="clsp", bufs=1, space="PSUM") as clp,
                tc.tile_pool(name="ep", bufs=3) as ep,
                tc.tile_pool(name="csb", bufs=2) as csp,
                tc.tile_pool(name="trashp", bufs=1) as trp,
            ):
                trash = trp.tile([P, W_STT], bf16, name="trash_stt")
                trash2 = trp.tile([P, HW2], bf16, name="trash_ts")

                def flush_cls(item):
                    # colsum matmuls delayed one job so the PE never waits
                    # on the exp of the job it just multiplied. lhsT is
                    # zero-padded to M=128 so the PE stays in 128x128
                    # tiling mode (smaller M switches tiling modes, which
                    # drains the array and broke partition-offset output).
                    (clsAB, clsS_t, e_t, jid_, G_, kind_, chunks, flags) = item
                    for bi, gk in enumerate(chunks):
                        st, sp = flags[gk]
                        cls_t = clsAB[gk // 2]
                        nc.tensor.matmul(
                            out=cls_t[:, (gk % 2) * MM_N : (gk % 2 + 1) * MM_N],
                            lhsT=ycl[:, jid_ * P : (jid_ + 1) * P],
                            rhs=e_t[:, bi * MM_N : (bi + 1) * MM_N],
                            start=st,
                            stop=sp,
                            skip_group_check=True,
                        )
                    # evacuate a bank pair the moment its last writer ran:
                    # A closes banks 0/1, D closes banks 2/3 (all G). The
                    # pairs are separate PSUM tiles so this copy never
                    # blocks the other pair's matmuls.
                    if kind_ in ("A", "D"):
                        pi = 0 if kind_ == "A" else 1
                        sl = slice(pi * HW2, (pi + 1) * HW2)
                        nc.vector.tensor_copy(
                            out=clsS_t[0:4, sl], in_=clsAB[pi][0:4, :]
                        )
                        nc.sync.dma_start(
                            out=cls_h[:, G_ * CH + sl.start : G_ * CH + sl.stop],
                            in_=clsS_t[0:4, sl],
                        )

                jid = 0
                for G in (3, 2, 1, 0):
                    nfull = [0, 2, 4, 6][G]
                    njobs = nfull + 3
                    kinds = ["full"] * nfull + ["B", "A", "D"]
                    # per-bank contributor order for start/stop flags
                    bank_seq = {k: [] for k in range(4)}
                    for s, kind in enumerate(kinds):
                        bk = ([0, 1, 2, 3] if kind == "full"
                              else [0, 1] if kind == "A" else [2, 3])
                        for k in bk:
                            bank_seq[k].append(s)
                    pending_cls = []
                    clsA = clp.tile([P, HW2], f32, tag="clsA", name=f"clsA{G}")
                    clsB = clp.tile([P, HW2], f32, tag="clsB", name=f"clsB{G}")
                    clsAB = (clsA, clsB)
                    clsS = csp.tile([P, CH], f32, tag="clsS", name=f"clsS{G}")
                    for s in range(njobs):
                        kind = kinds[s]
                        if kind == "full":
                            chunks = [0, 1, 2, 3]
                            width = CH
                            coff = 0
                        else:
                            chunks = [0, 1] if kind == "A" else [2, 3]
                            width = HW2
                            coff = 0 if kind == "A" else HW2
                        e = ep.tile([P, width], bf16,
                                    tag="e" if width == CH else "e1",
                                    name=f"e{jid}")
                        nhalf = width // HW2
                        for h in range(nhalf):
                            ps = mpp.tile([P, HW2], f32, tag="ps",
                                          name=f"ps{jid}_{h}")
                            for k in range(2):
                                c0 = coff + h * HW2 + k * MM_N
                                nc.tensor.matmul(
                                    out=ps[:, k * MM_N : (k + 1) * MM_N],
                                    lhsT=xiT[:, jid * P : (jid + 1) * P],
                                    rhs=xnT[G][:, c0 : c0 + MM_N],
                                    start=True,
                                    stop=True,
                                )
                            # down (row side): Act accum on h0/B halves
                            # (each accum costs a ~300ns read on the
                            # bottleneck engine); full h1 halves reduce on
                            # the DVE below. A/D rows get down from
                            # colsums by square symmetry.
                            acc = None
                            if kind == "full" and h == 0:
                                acc = outs[:, NJOBS + 2 * jid + h :
                                           NJOBS + 2 * jid + h + 1]
                            nc.scalar.activation(
                                out=e[:, h * HW2 : (h + 1) * HW2],
                                in_=ps[:],
                                func=AF.Exp,
                                scale=1.0 / TAU,
                                accum_out=acc,
                            )
                        # top (row side): masked sum over the rotated band
                        if kind == "full":
                            nc.vector.scalar_tensor_tensor(
                                out=trash[:, : POKE1 - POKE0],
                                in0=ybc[G][:, POKE0:POKE1],
                                scalar=yis[:, jid : jid + 1],
                                in1=e[:, POKE0:POKE1],
                                op0=OP.is_equal,
                                op1=OP.mult,
                                accum_out=outs[:, jid : jid + 1],
                            )
                        else:
                            nc.vector.scalar_tensor_tensor(
                                out=trash[:, :W_STT],
                                in0=(ybc[G][:, :W_STT] if kind == "A"
                                     else ybc[G][:, W_STT : 2 * W_STT]),
                                scalar=yis[:, jid : jid + 1],
                                in1=e[:, :W_STT],
                                op0=OP.is_equal,
                                op1=OP.mult,
                                accum_out=outs[:, jid : jid + 1],
                            )
                        # full h1 + B down on DVE (tensor_scalar reduce)
                        if kind in ("full", "B"):
                            dc = NJOBS + 2 * jid + (1 if kind == "full" else 0)
                            nc.vector.tensor_scalar(
                                out=trash2[:],
                                in0=e[:, HW2:CH] if kind == "full" else e[:],
                                scalar1=1.0,
                                scalar2=0.0,
                                op0=OP.mult,
                                op1=OP.add,
                                accum_out=outs[:, dc : dc + 1],
                            )
                        flags = {
                            gk: (bank_seq[gk][0] == s, bank_seq[gk][-1] == s)
                            for gk in chunks
                        }
                        pending_cls.append(
                            (clsAB, clsS, e, jid, G, kind, chunks, flags))
                        if pending_cls and (s >= 1 or G == 0):
                            flush_cls(pending_cls.pop(0))
                        jid += 1
                    while pending_cls:
                        flush_cls(pending_cls.pop(0))
                assert jid == NJOBS
            nc.scalar.dma_start(out=out_h[:, :], in_=outs[:])
    nc.compile()
    return nc


def _get_program():
    global _PROGRAM
    if _PROGRAM is None:
        _PROGRAM = _build_program()
    return _PROGRAM


def _prep(y):
    """Sorted order, class extents, straddle patches. Shared by
    make_in_maps and finalize."""
    y = np.asarray(y).astype(np.int64)
    perm = np.argsort(y, kind="stable")
    ys = y[perm]
    counts = np.bincount(ys, minlength=100)
    assert counts.max() <= P, f"class too large for band scheme: {counts.max()}"
    starts = {}
    ends = {}
    for c in np.unique(ys):
        idx = np.nonzero(ys == c)[0]
        starts[int(c)], ends[int(c)] = int(idx[0]), int(idx[-1] + 1)

    def straddle(b):
        for c, s0 in starts.items():
            if s0 < b < ends[c]:
                return c
        return None

    patches = []  # (G, kind, cls, boundary)
    for G in range(NG):
        if G > 0:
            c = straddle(2048 * G)
            if c is not None:
                patches.append((G, "k0", c, 2048 * G))
        c = straddle(2048 * G + 1024)
        if c is not None:
            patches.append((G, "k8", c, 2048 * G + 1024))
    return perm, ys, patches


def _rotate_halves(arr, w0):
    """Rotate each 1024-col half of each 2048-col group left by w0.
    arr: [rows, N]."""
    out = np.empty_like(arr)
    idx = (np.arange(HW2) + w0) % HW2
    for g in range(NG):
        for h in range(2):
            c0 = g * CH + h * HW2
            out[:, c0 : c0 + HW2] = arr[:, c0 + idx]
    return out


def make_in_maps(x, y):
    x = np.asarray(x, dtype=np.float64)
    perm, ys, patches = _prep(y)
    xs = x[perm]
    yf = ys.astype(np.float32)
    xn = xs / np.linalg.norm(xs, axis=-1, keepdims=True)
    xnT0 = np.ascontiguousarray(xn.T.astype(np.float32))   # [D, N]
    ybc0 = np.broadcast_to(yf[None, :], (P, N)).astype(np.float32)
    pos = np.arange(N)
    in_maps = []
    for core in range(NCORES):
        w0 = rot0(core)
        xnT = np.ascontiguousarray(
            _rotate_halves(xnT0, w0).astype(ml_dtypes.bfloat16))
        ybr = _rotate_halves(ybc0[0:1], w0)[0]          # [N] rotated classes
        # ship only the 384-wide STT windows: (g, half) -> first 384 cols
        ybw = np.empty((P, NG * 2 * W_STT), np.float32)
        for g in range(NG):
            for h in range(2):
                w = ybr[g * CH + h * HW2 : g * CH + h * HW2 + W_STT]
                ybw[:, (2 * g + h) * W_STT : (2 * g + h + 1) * W_STT] = w[None, :]
        jobs = job_table(core)
        xiT = np.empty((P, NJOBS * P), ml_dtypes.bfloat16)
        yis = np.empty((P, NJOBS), np.float32)
        ypt = np.zeros((P, NJOBS * 2), np.float32)
        for jid, (rho, G, kind, half) in enumerate(jobs):
            rows = slice(rho * P, (rho + 1) * P)
            xiT[:, jid * P : (jid + 1) * P] = xnT0[:, rows].astype(
                ml_dtypes.bfloat16)
            yis[:, jid] = yf[rows]
            if core == 7:
                rr = pos[rows]
                for (pG, pk, c, b) in patches:
                    if (pk == "k0" and kind == "full" and G == pG
                            and rho == 16 * pG - 1):
                        ypt[:, jid * 2 + 0] = (
                            (ys[rows] == c) & (rr < b)
                        ).astype(np.float32)
                    if pk == "k8" and kind == "B" and G == pG:
                        ypt[:, jid * 2 + 1] = (
                            (ys[rows] == c) & (rr < b)
                        ).astype(np.float32)
        in_maps.append(
            {
                "xnT": xnT,
                "xiT": np.ascontiguousarray(xiT),
                "ypatch": np.ascontiguousarray(ypt.astype(ml_dtypes.bfloat16)),
                "y_bcast": np.ascontiguousarray(ybw.astype(ml_dtypes.bfloat16)),
                "y_own": yis,
            }
        )
    return in_maps


def finalize(per_core_outs, per_core_cls, y):
    perm, ys, patches = _prep(y)
    pos = np.arange(N)
    down = np.zeros(N, np.float64)
    top = np.zeros(N, np.float64)
    for core in range(NCORES):
        w0 = rot0(core)
        o = np.asarray(per_core_outs[core], dtype=np.float64)  # [P, 3*NJOBS]
        cl = np.asarray(per_core_cls[core], dtype=np.float64)  # [4, NG*2048]
        for jid, (rho, G, kind, half) in enumerate(job_table(core)):
            rows = slice(rho * P, (rho + 1) * P)
            top[rows] += o[:, jid]
            if kind == "full":
                down[rows] += o[:, NJOBS + 2 * jid]
                down[rows] += o[:, NJOBS + 2 * jid + 1]
            elif kind == "B":
                down[rows] += o[:, NJOBS + 2 * jid]
        # unscramble this core's rotated colsum positions
        idx = (np.arange(HW2) + w0) % HW2
        for G in range(NG):
            for h in range(2):
                cols = 2048 * G + 1024 * h + idx       # actual columns
                down[cols] += cl[0, G * CH + h * HW2 : G * CH + (h + 1) * HW2]
        if core == 7:
            for (pG, pk, c, b) in patches:
                tgt = pos[(ys == c) & (pos >= b)]
                prow = 1 if pk == "k0" else 2
                for t in tgt:
                    h = (t - 2048 * pG) // HW2
                    rp = (t - 2048 * pG - HW2 * h - w0) % HW2
                    top[t] += cl[prow, pG * CH + h * HW2 + rp]
    return np.float32(np.mean(np.log(down) - np.log(top)))


def kernel(x, y):
    from concourse.bass_utils import run_bass_kernel_spmd

    nc = _get_program()
    in_maps = make_in_maps(x, y)
    res = run_bass_kernel_spmd(nc, in_maps, list(range(NCORES)))
    return finalize(
        [r["out"] for r in res.results],
        [r["cls_out"] for r in res.results],
        y,
    )
